# revision 34
# baseline (speedup 1.0000x reference)
"""DIEN (attention + AUGRU scan) Trainium2 Bass kernel.

Problem shapes: B=512, T=256, D=256, H=256.
Sharding: data-parallel over batch across 8 cores (64 rows/core), params
replicated. The AUGRU recurrence is computed to t=T; because masked
timesteps have softmax score exactly 0, h freezes past seq_len, so the
final h equals gru_out[b, seq_len[b]-1].

Internal gate order is [r | u | z] (reference order is [u | r | z]).
"""

import dataclasses
from contextlib import ExitStack

import numpy as np

import concourse.bass as bass
import concourse.mybir as mybir
import concourse.tile as tile
from concourse import bacc
from concourse.bass_utils import run_bass_kernel_spmd
from concourse.masks import make_identity

F32 = mybir.dt.float32
F32R = mybir.dt.float32r
I32 = mybir.dt.int32
AF = mybir.ActivationFunctionType
OP = mybir.AluOpType

NEG_INF = -2.0**32 + 1.0

B, T_FULL, D, H = 512, 256, 256, 256
N_CORES = 8
B_LOC = B // N_CORES  # 64
G3 = 3 * H  # 768


def _sl(ap, dim, step, count, elem_off):
    """Strided view of free dim `dim` (1-based into ap.ap) of an AP."""
    new = [list(p) for p in ap.ap]
    base_step = new[dim][0]
    new[dim] = [base_step * step, count]
    return dataclasses.replace(
        ap, ap=[tuple(p) for p in new], offset=ap.offset + ap.ap[dim][0] * elem_off
    )


def build_program(T=T_FULL, Tb=16, mm_dt=F32R, trail_blocks=2, stage="full"):
    """Build the Bass program for one core (B_LOC rows, T timesteps)."""
    nc = bacc.Bacc(None, target_bir_lowering=False)

    q_d = nc.dram_tensor("query", [B_LOC, D], F32, kind="ExternalInput")
    k_d = nc.dram_tensor("keys", [B_LOC, T, D], F32, kind="ExternalInput")
    sl_d = nc.dram_tensor("seq_len", [B_LOC, 1], I32, kind="ExternalInput")
    wa_d = nc.dram_tensor("w_att", [D, D], F32, kind="ExternalInput")
    w_d = nc.dram_tensor("w", [G3, D], F32, kind="ExternalInput")
    u_d = nc.dram_tensor("u", [G3, H], F32, kind="ExternalInput")
    bu_d = nc.dram_tensor("bu", [1, H], F32, kind="ExternalInput")
    br_d = nc.dram_tensor("br", [1, H], F32, kind="ExternalInput")
    bh_d = nc.dram_tensor("bh", [1, H], F32, kind="ExternalInput")
    out_d = nc.dram_tensor("out", [B_LOC, H], F32, kind="ExternalOutput")

    n_blocks = T // Tb
    n_mt = Tb // 2  # M-tiles (t-pairs) per block
    R = lambda ap: ap.bitcast(mm_dt)

    with tile.TileContext(nc) as tc, ExitStack() as ctx:
        const = ctx.enter_context(tc.tile_pool(name="const", bufs=1))
        kpool = ctx.enter_context(tc.tile_pool(name="kst", bufs=6))
        ixpool = ctx.enter_context(tc.tile_pool(name="ixbuf", bufs=trail_blocks + 1))
        step_p = ctx.enter_context(tc.tile_pool(name="step", bufs=2))
        pers = ctx.enter_context(tc.tile_pool(name="persist", bufs=1))
        ps_scan = ctx.enter_context(tc.tile_pool(name="ps_scan", bufs=2, space="PSUM"))
        ps_ix = ctx.enter_context(tc.tile_pool(name="ps_ix", bufs=1, space="PSUM"))

        # ---------------- constants ----------------
        uT = const.tile([128, 2, G3], F32R)  # [d_k, kk, (r|u|z)]
        wT = const.tile([128, 2, G3], F32R)
        wa = const.tile([128, 2, D], F32R)
        qT = const.tile([128, 2, B_LOC], F32R)
        qpT = const.tile([128, 2, B_LOC], F32R)
        i64s = const.tile([128, B_LOC], F32R)  # stacked I64: [p, c] = (p%64==c)
        i64f = const.tile([B_LOC, B_LOC], F32)  # f32 identity (transposes)
        i64sf = const.tile([128, B_LOC], F32)  # f32 stacked identity (ttr mask)
        ones_f = const.tile([1, 128], F32)
        ones_r = const.tile([1, 128], F32R)
        bias_r = const.tile([1, G3], F32R)
        iota_g = const.tile([B_LOC, T], F32)
        iota_t = const.tile([B_LOC, T], F32)
        neg_inf = const.tile([B_LOC, T], F32)
        seq_sb = const.tile([B_LOC, 1], I32)
        seq_f = const.tile([B_LOC, 1], F32)
        lg_cols = const.tile([128, T // 2], F32)
        lg_tmp = const.tile([B_LOC, T // 2], F32)
        logits = const.tile([B_LOC, T], F32)
        logits_m = const.tile([B_LOC, T], F32)
        exps = const.tile([B_LOC, T], F32)
        score = const.tile([B_LOC, T], F32)
        nmax = const.tile([B_LOC, 1], F32)
        sumexp = const.tile([B_LOC, 1], F32)
        recd = const.tile([B_LOC, 1], F32)

        make_identity(nc, i64f[:, :])
        make_identity(nc, i64sf[0:B_LOC, :])
        make_identity(nc, i64sf[B_LOC:128, :])
        # f32r tiles can't be memset/affine_select'd directly; round via ACT
        nc.scalar.copy(out=i64s[:, :], in_=i64sf[:, :])
        nc.vector.memset(ones_f[:, :], 1.0)
        nc.scalar.copy(out=ones_r[:, :], in_=ones_f[:, :])
        nc.vector.memset(neg_inf[:, :], NEG_INF)
        nc.gpsimd.iota(
            iota_g[:, :], pattern=[[1, T]], base=0, channel_multiplier=0,
            allow_small_or_imprecise_dtypes=True,
        )
        # bounce through DVE: TensorCopy can carry multiple sync waits, the
        # tensor_scalar (TS struct) below can only carry one
        nc.vector.tensor_copy(out=iota_t[:, :], in_=iota_g[:, :])
        nc.sync.dma_start(out=seq_sb[:, :], in_=sl_d[:, :])
        nc.vector.tensor_copy(out=seq_f[:, :], in_=seq_sb[:, :])

        # gate slot order [r, u, z] <- reference rows [u(0:H), r(H:2H), z(2H:3H)]
        slot_rows = [(0, H, H), (1, 0, H), (2, 2 * H, H)]  # (slot, row0, n)
        uT_dram = u_d[:, :].rearrange("g d -> d g")
        wT_dram = w_d[:, :].rearrange("g d -> d g")
        for kk in range(2):
            for slot, r0, n in slot_rows:
                nc.sync.dma_start(
                    out=uT[:, kk, slot * H : slot * H + n],
                    in_=uT_dram[kk * 128 : (kk + 1) * 128, r0 : r0 + n].bitcast(F32R),
                )
                nc.sync.dma_start(
                    out=wT[:, kk, slot * H : slot * H + n],
                    in_=wT_dram[kk * 128 : (kk + 1) * 128, r0 : r0 + n].bitcast(F32R),
                )
            nc.sync.dma_start(
                out=wa[:, kk, :],
                in_=wa_d[:, :].rearrange("i j -> j i")[kk * 128 : (kk + 1) * 128, :].bitcast(F32R),
            )
            nc.sync.dma_start(
                out=qT[:, kk, :],
                in_=q_d[:, :].rearrange("b d -> d b")[kk * 128 : (kk + 1) * 128, :].bitcast(F32R),
            )
        for slot, b_dram in [(0, br_d), (1, bu_d), (2, bh_d)]:
            nc.sync.dma_start(
                out=bias_r[0:1, slot * H : (slot + 1) * H], in_=b_dram[:, :].bitcast(F32R)
            )

        # ---------------- q_proj^T = (query @ w_att.T)^T ----------------
        # qpT[i, b] = sum_j w_att[i, j] * query[b, j]
        for mi in range(2):  # output i-tile
            ps_qp = ps_scan.tile([128, B_LOC], F32, tag="g")
            for kk in range(2):
                nc.tensor.matmul(
                    ps_qp[:, :],
                    R(wa[:, kk, mi * 128 : (mi + 1) * 128]),
                    R(qT[:, kk, :]),
                    start=(kk == 0),
                    stop=(kk == 1),
                )
            nc.scalar.copy(out=qpT[:, mi, :], in_=ps_qp[:, :])

        if stage == "qp":
            nc.sync.dma_start(
                out=out_d[0:B_LOC, 0:2 * B_LOC],
                in_=qpT[0:B_LOC, :, :].rearrange("p k b -> p (k b)").bitcast(F32),
            )
        # ---------------- keysT M-tile helpers ----------------
        kT_dram = k_d[:, :, :].rearrange("b t d -> d t b")

        def load_keyst(t0):
            """Load keys^T for t-pair (t0, t0+1): [128, kk, 2, 64]."""
            kst = kpool.tile([128, 2, 2, B_LOC], F32R, tag="kst")
            for kk in range(2):
                for toff in range(2):
                    nc.sync.dma_start(
                        out=kst[:, kk, toff, :],
                        in_=kT_dram[kk * 128 : (kk + 1) * 128, t0 + toff, :].bitcast(F32R),
                    )
            return kst

        # ---------------- attention logits pass ----------------
        # lhsT = keysT tile [d, (t,b)], rhs = qpT [d, b'] -> out[(t,b), b']
        # diagonal (b'==b) extracted via mask-multiply + free-dim reduce.
        for m in range(T // 2 if stage != "qp" else 0):
            kst = load_keyst(2 * m)
            ps_at = ps_scan.tile([128, B_LOC], F32, tag="g")
            for kk in range(2):
                nc.tensor.matmul(
                    ps_at[:, :],
                    R(kst[:, kk, :, :].rearrange("k t b -> k (t b)")),
                    R(qpT[:, kk, :]),
                    start=(kk == 0),
                    stop=(kk == 1),
                )
            scr = step_p.tile([128, B_LOC], F32, tag="scr")
            if stage == "attnmm":
                nc.scalar.copy(out=scr[:, :], in_=ps_at[:, :])
            else:
                nc.vector.scalar_tensor_tensor(
                    out=scr[:, :],
                    in0=ps_at[:, :],
                    scalar=1.0,
                    in1=i64sf[:, :],
                    op0=OP.bypass,
                    op1=OP.mult,
                    accum_out=lg_cols[:, m : m + 1],
                )

        # assemble logits [b, t]: lg_cols[p=(toff,b), m] = logits[b, 2m+toff]
        if stage not in ("qp", "lgcols", "attnmm"):
            nc.sync.dma_start(out=lg_tmp[:, :], in_=lg_cols[B_LOC:128, :])
            nc.vector.tensor_copy(out=_sl(logits[:, :], 1, 2, T // 2, 0), in_=lg_cols[0:B_LOC, :])
            nc.vector.tensor_copy(out=_sl(logits[:, :], 1, 2, T // 2, 1), in_=lg_tmp[:, :])

        if stage == "logits":
            nc.sync.dma_start(out=out_d[:, 0:T], in_=logits[:, :])
        if stage == "attnmm":
            nc.sync.dma_start(out=out_d[:, 0:B_LOC], in_=scr[0:B_LOC, :])
        if stage == "lgcols":
            nc.sync.dma_start(out=out_d[:, 0 : T // 2], in_=lg_cols[0:B_LOC, :])
            nc.sync.dma_start(out=out_d[:, T // 2 : T], in_=lg_cols[B_LOC:128, :])

        if stage not in ("qp", "logits", "lgcols", "attnmm"):
            # mask + softmax
            mask = step_p.tile([B_LOC, T], I32, tag="mask")
            nc.vector.tensor_scalar(
                out=mask[:, :], in0=iota_t[:, :], scalar1=seq_f[:, :], scalar2=None,
                op0=OP.is_lt,
            )
            nc.vector.select(
                out=logits_m[:, :], mask=mask[:, :], on_true=logits[:, :],
                on_false=neg_inf[:, :],
            )
            nc.vector.tensor_reduce(
                out=nmax[:, :], in_=logits_m[:, :], axis=mybir.AxisListType.X,
                op=OP.max, negate=True,
            )
            nc.scalar.activation(
                out=exps[:, :], in_=logits_m[:, :], func=AF.Exp,
                bias=nmax[:, :], scale=1.0, accum_out=sumexp[:, :],
            )
            nc.vector.reciprocal(out=recd[:, :], in_=sumexp[:, :])
            nc.vector.tensor_scalar_mul(score[:, :], exps[:, :], recd[:, :])

        if stage == "attn":
            nc.sync.dma_start(out=out_d[:, 0:T], in_=score[:, :])
        # ---------------- ix blocks ----------------
        run_scan = stage == "full"
        ix_tiles = [None] * n_blocks

        def emit_ix_mtile(blk, j):
            """ix[:, t0+2j : t0+2j+2, :] = keys @ w.T + bias, [128=(toff,b), G3]."""
            if j == 0:
                ix_tiles[blk] = ixpool.tile(
                    [128, n_mt, G3], F32R, tag="ix", name=f"ix_blk{blk}"
                )
            t0 = blk * Tb + 2 * j
            kst = load_keyst(t0)
            ixt = ix_tiles[blk]
            for half in range(2):
                ps = ps_ix.tile([128, G3 // 2], F32, tag=f"ix{half}")
                c0 = half * (G3 // 2)
                for kk in range(2):
                    nc.tensor.matmul(
                        ps[:, :],
                        R(kst[:, kk, :, :].rearrange("k t b -> k (t b)")),
                        R(wT[:, kk, c0 : c0 + G3 // 2]),
                        start=(kk == 0),
                        stop=False,
                    )
                nc.tensor.matmul(
                    ps[:, :],
                    R(ones_r[:, :]),
                    R(bias_r[:, c0 : c0 + G3 // 2]),
                    start=False,
                    stop=True,
                )
                nc.scalar.copy(out=ixt[:, j, c0 : c0 + G3 // 2], in_=ps[:, :])

        for blk in range(min(trail_blocks, n_blocks) if run_scan else 0):
            for j in range(n_mt):
                emit_ix_mtile(blk, j)

        # ---------------- AUGRU scan ----------------
        h_sb = pers.tile([B_LOC, H], F32)
        hT = pers.tile([128, 2, B_LOC], F32R)
        zeros_f = pers.tile([128, 128], F32)
        nc.vector.memset(h_sb[:, :], 0.0)
        nc.vector.memset(zeros_f[:, :], 0.0)
        nc.scalar.copy(
            out=hT[:, :, :].rearrange("p k b -> p (k b)"), in_=zeros_f[:, :]
        )

        for t in range(T if run_scan else 0):
            blk, jj = t // Tb, t % Tb
            # prefetch ix blocks `trail_blocks` ahead
            nb = blk + trail_blocks
            if nb < n_blocks and jj % 2 == 0 and jj // 2 < n_mt:
                emit_ix_mtile(nb, jj // 2)

            ixt = ix_tiles[blk]
            poff = (jj % 2) * B_LOC  # partition base of ix row-slice
            ix_row = ixt[poff : poff + B_LOC, jj // 2, :]  # [64, G3]

            # merged gate PSUM [64, 1024]: bank0 = r (cols 0:256, rest pad),
            # bank1 = u (512:768) + z (768:1024). sigma_r only touches bank0,
            # so it is not serialized against the u/z matmul writes.
            ps_g = ps_scan.tile([B_LOC, 1024], F32, tag="g")
            ps_r = ps_g[:, 0:H]
            ps_u = ps_g[:, 512 : 512 + H]
            ps_z = ps_g[:, 768 : 768 + H]
            # ix_z shifted to partitions 0:64 via identity matmul (shares the
            # Cf bank slot; lifetimes don't overlap within a step)
            ps_iz = ps_scan.tile([B_LOC, H], F32, tag="cf")
            nc.tensor.matmul(
                ps_iz[:, :],
                R(i64s[poff : poff + B_LOC, :]),
                R(ix_row[:, 2 * H : 3 * H]),
                start=True, stop=True,
            )
            # r gate first (it heads the serial chain)
            for slot, ps, with_ix in ((0, ps_r, True), (1, ps_u, True), (2, ps_z, False)):
                c0 = slot * H
                nc.tensor.matmul(
                    ps[:, :], R(hT[:, 0, :]), R(uT[:, 0, c0 : c0 + H]),
                    start=True, stop=False,
                )
                nc.tensor.matmul(
                    ps[:, :], R(hT[:, 1, :]), R(uT[:, 1, c0 : c0 + H]),
                    start=False, stop=not with_ix,
                )
                if with_ix:
                    nc.tensor.matmul(
                        ps[:, :],
                        R(i64s[poff : poff + B_LOC, :]),
                        R(ix_row[:, c0 : c0 + H]),
                        start=False, stop=True,
                    )

            r_g = step_p.tile([B_LOC, H], F32, tag="r")
            gu = step_p.tile([B_LOC, H], F32, tag="gu")
            q_g = step_p.tile([B_LOC, H], F32, tag="q")
            sz = step_p.tile([B_LOC, H], F32, tag="sz")
            z_g = step_p.tile([B_LOC, H], F32, tag="z")
            d_g = step_p.tile([B_LOC, H], F32, tag="d")
            e_g = step_p.tile([B_LOC, H], F32, tag="e")

            nc.scalar.activation(out=r_g[:, :], in_=ps_r[:, :], func=AF.Sigmoid)
            nc.scalar.activation(out=gu[:, :], in_=ps_u[:, :], func=AF.Sigmoid)
            nc.vector.tensor_mul(q_g[:, :], ps_z[:, :], r_g[:, :])
            nc.vector.tensor_add(sz[:, :], q_g[:, :], ps_iz[:, :])
            nc.scalar.activation(out=z_g[:, :], in_=sz[:, :], func=AF.Tanh)
            nc.vector.tensor_sub(d_g[:, :], z_g[:, :], h_sb[:, :])
            nc.vector.scalar_tensor_tensor(
                out=e_g[:, :], in0=gu[:, :], scalar=score[:, t : t + 1],
                in1=d_g[:, :], op0=OP.mult, op1=OP.mult,
            )
            nc.vector.tensor_add(h_sb[:, :], h_sb[:, :], e_g[:, :])
            # hT += transpose(e)
            ps_f = ps_scan.tile([128, 2, B_LOC], F32, tag="cf")
            for half in range(2):
                nc.tensor.transpose(
                    ps_f[:, half, :],
                    e_g[:, half * 128 : (half + 1) * 128],
                    i64f[:, :],
                )
            nc.vector.tensor_add(
                hT[:, :, :].rearrange("p k b -> p (k b)"),
                ps_f[:, :, :].rearrange("p k b -> p (k b)"),
                hT[:, :, :].rearrange("p k b -> p (k b)"),
            )

        if run_scan:
            nc.sync.dma_start(out=out_d[:, :], in_=h_sb[:, :])

    nc.finalize()
    return nc


def _shard_inputs(query, keys, seq_len, w_att, w, u, bu, br, bh, T=T_FULL):
    in_maps = []
    for c in range(N_CORES):
        s = slice(c * B_LOC, (c + 1) * B_LOC)
        in_maps.append(
            {
                "query": np.ascontiguousarray(query[s], dtype=np.float32),
                "keys": np.ascontiguousarray(keys[s, :T], dtype=np.float32),
                "seq_len": np.ascontiguousarray(
                    seq_len[s].reshape(B_LOC, 1), dtype=np.int32
                ),
                "w_att": np.ascontiguousarray(w_att, dtype=np.float32),
                "w": np.ascontiguousarray(w, dtype=np.float32),
                "u": np.ascontiguousarray(u, dtype=np.float32),
                "bu": np.ascontiguousarray(bu.reshape(1, -1), dtype=np.float32),
                "br": np.ascontiguousarray(br.reshape(1, -1), dtype=np.float32),
                "bh": np.ascontiguousarray(bh.reshape(1, -1), dtype=np.float32),
            }
        )
    return in_maps


_CACHED = {}


def run_on_device(inputs, T=T_FULL, Tb=16, trace=False, **build_kw):
    key = (T, Tb, tuple(sorted(build_kw.items())))
    if key not in _CACHED:
        _CACHED[key] = build_program(T=T, Tb=Tb, **build_kw)
    nc = _CACHED[key]
    in_maps = _shard_inputs(**inputs, T=T)
    res = run_bass_kernel_spmd(
        nc, in_maps, core_ids=list(range(N_CORES)), trace=trace
    )
    out = np.concatenate([r["out"] for r in res.results], axis=0)
    return out, res


def kernel(query, keys, seq_len, w_att, w, u, bu, br, bh):
    out, _ = run_on_device(
        dict(
            query=query, keys=keys, seq_len=seq_len, w_att=w_att, w=w, u=u,
            bu=bu, br=br, bh=bh,
        )
    )
    return out.astype(np.float32)


# revision 36
# speedup vs baseline: 9.1855x; 9.1855x over previous
"""DIEN (attention + AUGRU scan) Trainium2 Bass kernel.

Problem shapes: B=512, T=256, D=256, H=256.
Sharding: data-parallel over batch across 8 cores (64 rows/core), params
replicated. The AUGRU recurrence is computed to t=T; because masked
timesteps have softmax score exactly 0, h freezes past seq_len, so the
final h equals gru_out[b, seq_len[b]-1].

Keys are DMA'd in natural layout (contiguous 16KB/partition chunks) and
transposed on-chip by the PE; a strided DMA of keys^T is descriptor-rate
bound (512B descriptors) and ~100x slower in aggregate.

Internal gate order is [r | u | z] (reference order is [u | r | z]).
All matmuls run in float32r (full-rate fp32, ~1e-3 relative accuracy).
"""

from contextlib import ExitStack

import numpy as np

import concourse.bass as bass
import concourse.mybir as mybir
import concourse.tile as tile
from concourse import bacc
from concourse.bass_utils import run_bass_kernel_spmd
from concourse.masks import make_identity

F32 = mybir.dt.float32
F32R = mybir.dt.float32r
I32 = mybir.dt.int32
AF = mybir.ActivationFunctionType
OP = mybir.AluOpType

NEG_INF = -2.0**32 + 1.0

B, T_FULL, D, H = 512, 256, 256, 256
N_CORES = 8
B_LOC = B // N_CORES  # 64
G3 = 3 * H  # 768


def build_program(T=T_FULL, Tb=16, mm_dt=F32R, trail_blocks=2, stage="full"):
    """Build the Bass program for one core (B_LOC rows, T timesteps)."""
    nc = bacc.Bacc(None, target_bir_lowering=False)

    q_d = nc.dram_tensor("query", [B_LOC, D], F32, kind="ExternalInput")
    k_d = nc.dram_tensor("keys", [B_LOC, T, D], F32, kind="ExternalInput")
    sl_d = nc.dram_tensor("seq_len", [B_LOC, 1], I32, kind="ExternalInput")
    wa_d = nc.dram_tensor("w_att", [D, D], F32, kind="ExternalInput")
    w_d = nc.dram_tensor("w", [G3, D], F32, kind="ExternalInput")
    u_d = nc.dram_tensor("u", [G3, H], F32, kind="ExternalInput")
    bu_d = nc.dram_tensor("bu", [1, H], F32, kind="ExternalInput")
    br_d = nc.dram_tensor("br", [1, H], F32, kind="ExternalInput")
    bh_d = nc.dram_tensor("bh", [1, H], F32, kind="ExternalInput")
    out_d = nc.dram_tensor("out", [B_LOC, H], F32, kind="ExternalOutput")

    n_blocks = T // Tb
    n_mt = Tb // 2  # M-tiles (t-pairs) per block
    R = lambda ap: ap.bitcast(mm_dt)

    with tile.TileContext(nc) as tc, ExitStack() as ctx:
        const = ctx.enter_context(tc.tile_pool(name="const", bufs=1))
        kbpool = ctx.enter_context(tc.tile_pool(name="kb", bufs=3))
        kpool = ctx.enter_context(tc.tile_pool(name="kst", bufs=4))
        ixpool = ctx.enter_context(tc.tile_pool(name="ixbuf", bufs=trail_blocks + 1))
        step_p = ctx.enter_context(tc.tile_pool(name="step", bufs=2))
        pers = ctx.enter_context(tc.tile_pool(name="persist", bufs=1))
        ps_scan = ctx.enter_context(tc.tile_pool(name="ps_scan", bufs=1, space="PSUM"))
        ps_ix = ctx.enter_context(tc.tile_pool(name="ps_ix", bufs=2, space="PSUM"))

        # ---------------- constants ----------------
        uT = const.tile([128, 2, G3], F32R)  # [d_k, kk, (r|u|z)]
        wT = const.tile([128, 2, G3], F32R)
        wa = const.tile([128, 2, D], F32R)
        qT = const.tile([128, 2, B_LOC], F32R)
        qp_sb = const.tile([B_LOC, D], F32)
        i64s = const.tile([128, B_LOC], F32R)  # stacked I64: [p, c] = (p%64==c)
        i64sf = const.tile([128, B_LOC], F32)  # f32 source for i64s
        i64f = const.tile([B_LOC, B_LOC], F32)  # f32 identity (transposes)
        ones_f = const.tile([1, 128], F32)
        ones_r = const.tile([1, 128], F32R)
        bias_r = const.tile([1, G3], F32R)
        iota_g = const.tile([B_LOC, T], F32)
        iota_t = const.tile([B_LOC, T], F32)
        neg_inf = const.tile([B_LOC, T], F32)
        seq_sb = const.tile([B_LOC, 1], I32)
        seq_f = const.tile([B_LOC, 1], F32)
        logits = const.tile([B_LOC, T], F32)
        logits_m = const.tile([B_LOC, T], F32)
        exps = const.tile([B_LOC, T], F32)
        score = const.tile([B_LOC, T], F32)
        nmax = const.tile([B_LOC, 1], F32)
        sumexp = const.tile([B_LOC, 1], F32)
        recd = const.tile([B_LOC, 1], F32)

        make_identity(nc, i64f[:, :])
        make_identity(nc, i64sf[0:B_LOC, :])
        make_identity(nc, i64sf[B_LOC:128, :])
        # f32r tiles can't be memset/affine_select'd directly; round via ACT
        nc.scalar.copy(out=i64s[:, :], in_=i64sf[:, :])
        nc.vector.memset(ones_f[:, :], 1.0)
        nc.scalar.copy(out=ones_r[:, :], in_=ones_f[:, :])
        nc.vector.memset(neg_inf[:, :], NEG_INF)
        nc.gpsimd.iota(
            iota_g[:, :], pattern=[[1, T]], base=0, channel_multiplier=0,
            allow_small_or_imprecise_dtypes=True,
        )
        # bounce through DVE: TensorCopy can carry multiple sync waits, the
        # tensor_scalar (TS struct) below can only carry one
        nc.vector.tensor_copy(out=iota_t[:, :], in_=iota_g[:, :])
        nc.sync.dma_start(out=seq_sb[:, :], in_=sl_d[:, :])
        nc.vector.tensor_copy(out=seq_f[:, :], in_=seq_sb[:, :])

        # gate slot order [r, u, z] <- reference rows [u(0:H), r(H:2H), z(2H:3H)]
        slot_rows = [(0, H, H), (1, 0, H), (2, 2 * H, H)]  # (slot, row0, n)
        uT_dram = u_d[:, :].rearrange("g d -> d g")
        wT_dram = w_d[:, :].rearrange("g d -> d g")
        for kk in range(2):
            for slot, r0, n in slot_rows:
                nc.sync.dma_start(
                    out=uT[:, kk, slot * H : slot * H + n],
                    in_=uT_dram[kk * 128 : (kk + 1) * 128, r0 : r0 + n].bitcast(F32R),
                )
                nc.sync.dma_start(
                    out=wT[:, kk, slot * H : slot * H + n],
                    in_=wT_dram[kk * 128 : (kk + 1) * 128, r0 : r0 + n].bitcast(F32R),
                )
            nc.sync.dma_start(
                out=wa[:, kk, :],
                in_=wa_d[:, :]
                .rearrange("i j -> j i")[kk * 128 : (kk + 1) * 128, :]
                .bitcast(F32R),
            )
            nc.sync.dma_start(
                out=qT[:, kk, :],
                in_=q_d[:, :]
                .rearrange("b d -> d b")[kk * 128 : (kk + 1) * 128, :]
                .bitcast(F32R),
            )
        for slot, b_dram in [(0, br_d), (1, bu_d), (2, bh_d)]:
            nc.sync.dma_start(
                out=bias_r[0:1, slot * H : (slot + 1) * H],
                in_=b_dram[:, :].bitcast(F32R),
            )

        # ---------------- q_proj = query @ w_att.T (batch-major) ----------
        ps_qp = ps_scan.tile([B_LOC, D], F32, tag="g")
        for kk in range(2):
            nc.tensor.matmul(
                ps_qp[:, :],
                R(qT[:, kk, :]),
                R(wa[:, kk, :]),
                start=(kk == 0),
                stop=(kk == 1),
            )
        nc.scalar.copy(out=qp_sb[:, :], in_=ps_qp[:, :])

        # ---------------- keys block loader (natural layout) --------------
        def load_kblock(blk, who):
            kb = kbpool.tile([B_LOC, Tb, D], F32, tag="kb", name=f"kb_{who}{blk}")
            nc.sync.dma_start(
                out=kb[:, :, :], in_=k_d[:, blk * Tb : (blk + 1) * Tb, :]
            )
            return kb

        # ---------------- attention logits ----------------
        # logits[b, t] = sum_d qp[b, d] * keys[b, t, d]  (per-partition accum)
        for blk in range(n_blocks):
            kb = load_kblock(blk, "a")
            for tau in range(Tb):
                t = blk * Tb + tau
                scr = step_p.tile([B_LOC, D], F32, tag="scr")
                nc.vector.scalar_tensor_tensor(
                    out=scr[:, :],
                    in0=kb[:, tau, :],
                    scalar=1.0,
                    in1=qp_sb[:, :],
                    op0=OP.bypass,
                    op1=OP.mult,
                    accum_out=logits[:, t : t + 1],
                )

        # ---------------- mask + softmax ----------------
        mask = step_p.tile([B_LOC, T], I32, tag="mask")
        nc.vector.tensor_scalar(
            out=mask[:, :], in0=iota_t[:, :], scalar1=seq_f[:, :], scalar2=None,
            op0=OP.is_lt,
        )
        nc.vector.select(
            out=logits_m[:, :], mask=mask[:, :], on_true=logits[:, :],
            on_false=neg_inf[:, :],
        )
        nc.vector.tensor_reduce(
            out=nmax[:, :], in_=logits_m[:, :], axis=mybir.AxisListType.X,
            op=OP.max, negate=True,
        )
        nc.scalar.activation(
            out=exps[:, :], in_=logits_m[:, :], func=AF.Exp,
            bias=nmax[:, :], scale=1.0, accum_out=sumexp[:, :],
        )
        nc.vector.reciprocal(out=recd[:, :], in_=sumexp[:, :])
        if stage == "scanonly":
            nc.vector.memset(score[:, :], 1.0 / T)
        else:
            nc.vector.tensor_scalar_mul(score[:, :], exps[:, :], recd[:, :])

        if stage == "attn":
            nc.sync.dma_start(out=out_d[:, 0:T], in_=score[:, :])

        # ---------------- ix blocks ----------------
        run_scan = stage in ("full", "scanonly")
        ix_tiles = [None] * n_blocks
        kb_tiles = [None] * n_blocks

        def emit_ix_mtile(blk, j):
            """ix[:, 2j:2j+2, :] = keys @ w.T + bias -> [128=(toff,b), G3].

            keys^T tiles are produced on-chip: PE-transpose the batch-major
            block, then one ACT copy PSUM->SBUF (rounding to f32r).
            """
            if j == 0:
                kb_tiles[blk] = load_kblock(blk, "x")
                ix_tiles[blk] = ixpool.tile(
                    [128, n_mt, G3], F32R, tag="ix", name=f"ix_blk{blk}"
                )
            kb = kb_tiles[blk]
            ixt = ix_tiles[blk]
            # transpose 2 timesteps x 2 d-halves: [64, 128] -> [128, 64]
            ktr = ps_scan.tile([128, 4, B_LOC], F32, tag="ktr")
            for kk in range(2):
                for toff in range(2):
                    nc.tensor.transpose(
                        ktr[:, kk * 2 + toff, :],
                        kb[:, 2 * j + toff, kk * 128 : (kk + 1) * 128],
                        i64f[:, :],
                    )
            kst = kpool.tile([128, 2, 2, B_LOC], F32R, tag="kst")
            nc.scalar.copy(
                out=kst[:, :, :, :].rearrange("p a b c -> p (a b c)"),
                in_=ktr[:, :, :].rearrange("p a b -> p (a b)"),
            )
            ixps = ps_ix.tile([128, G3], F32, tag="ixps")
            for c0, n_c in ((0, 512), (512, 256)):
                for kk in range(2):
                    nc.tensor.matmul(
                        ixps[:, c0 : c0 + n_c],
                        R(kst[:, kk, :, :].rearrange("k t b -> k (t b)")),
                        R(wT[:, kk, c0 : c0 + n_c]),
                        start=(kk == 0),
                        stop=False,
                    )
                nc.tensor.matmul(
                    ixps[:, c0 : c0 + n_c],
                    R(ones_r[:, :]),
                    R(bias_r[:, c0 : c0 + n_c]),
                    start=False,
                    stop=True,
                )
            nc.scalar.copy(out=ixt[:, j, :], in_=ixps[:, :])

        for blk in range(min(trail_blocks, n_blocks) if run_scan else 0):
            for j in range(n_mt):
                emit_ix_mtile(blk, j)

        # ---------------- AUGRU scan ----------------
        if run_scan:
            h_sb = pers.tile([B_LOC, H], F32)
            hT = pers.tile([128, 2, B_LOC], F32R)
            zeros_f = pers.tile([128, 128], F32)
            nc.vector.memset(h_sb[:, :], 0.0)
            nc.vector.memset(zeros_f[:, :], 0.0)
            nc.scalar.copy(
                out=hT[:, :, :].rearrange("p k b -> p (k b)"), in_=zeros_f[:, :]
            )

        for t in range(T if run_scan else 0):
            blk, jj = t // Tb, t % Tb
            # prefetch ix blocks `trail_blocks` ahead
            nb = blk + trail_blocks
            if nb < n_blocks and jj % 2 == 0 and jj // 2 < n_mt:
                emit_ix_mtile(nb, jj // 2)

            ixt = ix_tiles[blk]
            poff = (jj % 2) * B_LOC  # partition base of ix row-slice
            ix_row = ixt[poff : poff + B_LOC, jj // 2, :]  # [64, G3]

            # merged gate PSUM [64, 1024]: bank0 = r (cols 0:256, rest pad),
            # bank1 = u (512:768) + z (768:1024). sigma_r only touches bank0,
            # so it is not serialized against the u/z matmul writes.
            ps_g = ps_scan.tile([B_LOC, 1024], F32, tag="g")
            ps_r = ps_g[:, 0:H]
            ps_u = ps_g[:, 512 : 512 + H]
            ps_z = ps_g[:, 768 : 768 + H]
            # ix_z shifted to partitions 0:64 via identity matmul (shares the
            # cf bank slot; lifetimes don't overlap within a step)
            ps_iz = ps_scan.tile([B_LOC, H], F32, tag="cf")
            nc.tensor.matmul(
                ps_iz[:, :],
                R(i64s[poff : poff + B_LOC, :]),
                R(ix_row[:, 2 * H : 3 * H]),
                start=True,
                stop=True,
            )
            # r gate first (it heads the serial chain)
            for slot, ps, with_ix in ((0, ps_r, True), (1, ps_u, True), (2, ps_z, False)):
                c0 = slot * H
                nc.tensor.matmul(
                    ps[:, :], R(hT[:, 0, :]), R(uT[:, 0, c0 : c0 + H]),
                    start=True, stop=False,
                )
                nc.tensor.matmul(
                    ps[:, :], R(hT[:, 1, :]), R(uT[:, 1, c0 : c0 + H]),
                    start=False, stop=not with_ix,
                )
                if with_ix:
                    nc.tensor.matmul(
                        ps[:, :],
                        R(i64s[poff : poff + B_LOC, :]),
                        R(ix_row[:, c0 : c0 + H]),
                        start=False, stop=True,
                    )

            r_g = step_p.tile([B_LOC, H], F32, tag="r")
            gu = step_p.tile([B_LOC, H], F32, tag="gu")
            q_g = step_p.tile([B_LOC, H], F32, tag="q")
            sz = step_p.tile([B_LOC, H], F32, tag="sz")
            z_g = step_p.tile([B_LOC, H], F32, tag="z")
            d_g = step_p.tile([B_LOC, H], F32, tag="d")
            e_g = step_p.tile([B_LOC, H], F32, tag="e")

            nc.scalar.activation(out=r_g[:, :], in_=ps_r[:, :], func=AF.Sigmoid)
            nc.scalar.activation(out=gu[:, :], in_=ps_u[:, :], func=AF.Sigmoid)
            nc.vector.tensor_mul(q_g[:, :], ps_z[:, :], r_g[:, :])
            nc.vector.tensor_add(sz[:, :], q_g[:, :], ps_iz[:, :])
            nc.scalar.activation(out=z_g[:, :], in_=sz[:, :], func=AF.Tanh)
            nc.vector.tensor_sub(d_g[:, :], z_g[:, :], h_sb[:, :])
            nc.vector.scalar_tensor_tensor(
                out=e_g[:, :], in0=gu[:, :], scalar=score[:, t : t + 1],
                in1=d_g[:, :], op0=OP.mult, op1=OP.mult,
            )
            nc.vector.tensor_add(h_sb[:, :], h_sb[:, :], e_g[:, :])
            # hT += transpose(e)
            ps_f = ps_scan.tile([128, 2, B_LOC], F32, tag="cf")
            for half in range(2):
                nc.tensor.transpose(
                    ps_f[:, half, :],
                    e_g[:, half * 128 : (half + 1) * 128],
                    i64f[:, :],
                )
            nc.vector.tensor_add(
                hT[:, :, :].rearrange("p k b -> p (k b)"),
                ps_f[:, :, :].rearrange("p k b -> p (k b)"),
                hT[:, :, :].rearrange("p k b -> p (k b)"),
            )

        if run_scan:
            nc.sync.dma_start(out=out_d[:, :], in_=h_sb[:, :])

    nc.finalize()
    return nc


def _shard_inputs(query, keys, seq_len, w_att, w, u, bu, br, bh, T=T_FULL):
    in_maps = []
    for c in range(N_CORES):
        s = slice(c * B_LOC, (c + 1) * B_LOC)
        in_maps.append(
            {
                "query": np.ascontiguousarray(query[s], dtype=np.float32),
                "keys": np.ascontiguousarray(keys[s, :T], dtype=np.float32),
                "seq_len": np.ascontiguousarray(
                    seq_len[s].reshape(B_LOC, 1), dtype=np.int32
                ),
                "w_att": np.ascontiguousarray(w_att, dtype=np.float32),
                "w": np.ascontiguousarray(w, dtype=np.float32),
                "u": np.ascontiguousarray(u, dtype=np.float32),
                "bu": np.ascontiguousarray(bu.reshape(1, -1), dtype=np.float32),
                "br": np.ascontiguousarray(br.reshape(1, -1), dtype=np.float32),
                "bh": np.ascontiguousarray(bh.reshape(1, -1), dtype=np.float32),
            }
        )
    return in_maps


_CACHED = {}


def run_on_device(inputs, T=T_FULL, Tb=16, trace=False, **build_kw):
    key = (T, Tb, tuple(sorted(build_kw.items())))
    if key not in _CACHED:
        _CACHED[key] = build_program(T=T, Tb=Tb, **build_kw)
    nc = _CACHED[key]
    in_maps = _shard_inputs(**inputs, T=T)
    res = run_bass_kernel_spmd(
        nc, in_maps, core_ids=list(range(N_CORES)), trace=trace
    )
    out = np.concatenate([r["out"] for r in res.results], axis=0)
    return out, res


def kernel(query, keys, seq_len, w_att, w, u, bu, br, bh):
    out, _ = run_on_device(
        dict(
            query=query, keys=keys, seq_len=seq_len, w_att=w_att, w=w, u=u,
            bu=bu, br=br, bh=bh,
        )
    )
    return out.astype(np.float32)
